# revision 1
# baseline (speedup 1.0000x reference)
"""Multi-head causal attention (B=2, T=2048, E=1024, H=16, D=64) on 8 trn2 cores.

Sharding: tensor-parallel over heads — core c owns heads {2c, 2c+1} (a 128-wide
slice of the hidden dim). Each core computes q/k/v projections for its heads
over the full sequence, causal attention, and a partial output projection
(contraction over its 128 rows of Wo). The host sums the 8 partials + bias.

Per-core device program (SPMD — one NEFF, different weight slices per core):
  projections: QT/KT = (W.T @ xT) in [dim, token] layout (weight-stationary,
    token-moving N=512); V in natural [token, dim|1] layout via
    xT-chunk-stationary matmuls. The appended ones column makes the P@V
    matmul emit Z = sum(exp) as psum row 64 for free. Batch-1 projections are
    emitted in small units interleaved into batch-0's attention waves so the
    PE slack there absorbs them.
  attention, per (batch, 512-wide tq chunk), in waves of two 128-row tk
    blocks: S^T = K Q^T with both heads packed on the PE via row tiling
    (contraction rows 0-63 / 64-127 run concurrently), causally trimmed
    moving ranges; P^T = exp(0.125 * S^T) on ScalarE straight out of PSUM
    (safe without max-subtraction: scores ~ N(0,1), |s| < ~7); diagonal
    128x128 triangle masks multiplied into P^T on VectorE; O^T|Z = [V|1]^T
    P^T with causality-limited moving ranges; 1/Z (fp16) broadcast across
    the 64 head dims by a K=1 outer-product matmul; normalize on VectorE.
  output: out[tq, :] = O^T.T @ Wo_slice (K=128), copies + DMA per 128 rows.

Timing signal during development was concourse's TimelineSim cost model
(no NTFF profiling exists under this axon client); measured 164.4us per core,
engine busy: PE 118us, ScalarE 95us, VectorE 79us, DMA 73us. Weight DMAs are
queue-ordered so the first projection matmul's gates (wq + first xT pair)
land ahead of weights not needed until later.
"""

import os
import numpy as np
import ml_dtypes

import concourse.bass as bass
import concourse.tile as tile
from concourse import bacc, mybir
from concourse.bass_utils import run_bass_kernel_spmd
from contextlib import ExitStack

B, T, E, H, D = 2, 2048, 1024, 16, 64
BT = B * T            # 4096 tokens total
NCORE = 8
KC = E // 128         # contraction chunks for projections = 8
CQ = 512              # tq chunk width
NQB = T // CQ         # tq chunks per batch = 4
NKB = T // 128        # tk blocks per batch = 16

F32 = mybir.dt.float32
BF16 = mybir.dt.bfloat16
AF = mybir.ActivationFunctionType

_cache = {}


def _build():
    nc = bacc.Bacc("TRN2", target_bir_lowering=False, debug=False,
                   num_devices=NCORE)

    xT = nc.dram_tensor("xT", [E, BT], BF16, kind="ExternalInput").ap()
    wq = nc.dram_tensor("wq", [128, E], BF16, kind="ExternalInput").ap()
    wk = nc.dram_tensor("wk", [128, E], BF16, kind="ExternalInput").ap()
    wv = nc.dram_tensor("wv", [128, E], BF16, kind="ExternalInput").ap()
    wo = nc.dram_tensor("wo", [128, E], BF16, kind="ExternalInput").ap()
    tri = nc.dram_tensor("tri", [128, 128], BF16, kind="ExternalInput").ap()
    out = nc.dram_tensor("out", [BT, E], F32, kind="ExternalOutput").ap()

    with tile.TileContext(nc) as tc, ExitStack() as ctx:
        pers = ctx.enter_context(tc.tile_pool(name="pers", bufs=1))

        wq_sb = pers.tile([128, KC, 128], BF16, tag="wq")
        wk_sb = pers.tile([128, KC, 128], BF16, tag="wk")
        wv_sb = pers.tile([128, KC, 128], BF16, tag="wv")
        wo_sb = pers.tile([128, E], BF16, tag="wo")
        tri_sb = pers.tile([128, 128], BF16, tag="tri")
        ones_sb = pers.tile([128, 64], mybir.dt.float16, tag="ones")
        qt_sb = pers.tile([128, BT], BF16, tag="qt")    # [dims(2 heads), tok]
        kt_sb = pers.tile([128, BT], BF16, tag="kt")
        # V natural + ones col per head: [tok%128, blk, h, d|1]
        v_sb = pers.tile([128, BT // 128, 2, 65], BF16, tag="v")
        ot_sb = pers.tile([128, BT], BF16, tag="ot")    # attn out, [dims, tok]

        # wq/wk queued first on HWDGE; wv/tri/wo go after the first xT pair
        # (they are not needed until ~10us in) so the first projection
        # matmuls start as early as possible.
        nc.sync.dma_start(wq_sb[:], wq.rearrange("p (kc d) -> p kc d", kc=KC))
        nc.sync.dma_start(wk_sb[:], wk.rearrange("p (kc d) -> p kc d", kc=KC))
        nc.vector.memset(ones_sb[:], 1.0)
        nc.vector.memset(v_sb[:, :, :, 64:65], 1.0)

        def load_late_weights():
            nc.sync.dma_start(wv_sb[:],
                              wv.rearrange("p (kc d) -> p kc d", kc=KC))
            nc.sync.dma_start(tri_sb[:], tri[:])
            nc.sync.dma_start(wo_sb[:], wo[:])

        # Unified pools: PSUM tags share the 8 banks so batch-1 projections
        # overlap batch-0 attention, and attention waves start as soon as
        # their tk blocks are projected.
        xts_pool = ctx.enter_context(tc.tile_pool(name="xts", bufs=16))
        sc_pool = ctx.enter_context(tc.tile_pool(name="sc", bufs=2, space="PSUM"))
        pv_pool = ctx.enter_context(tc.tile_pool(name="pv", bufs=2, space="PSUM"))
        vps = ctx.enter_context(tc.tile_pool(name="vps", bufs=1, space="PSUM"))
        ops_pool = ctx.enter_context(tc.tile_pool(name="ops", bufs=1, space="PSUM"))
        pt_pool = ctx.enter_context(tc.tile_pool(name="pt", bufs=3))
        zr_pool = ctx.enter_context(tc.tile_pool(name="zr", bufs=3))
        zbs_pool = ctx.enter_context(tc.tile_pool(name="zbs", bufs=3))
        ost_pool = ctx.enter_context(tc.tile_pool(name="ost", bufs=6))

        def proj_pair_units(t0):
                xts = []
                for kc in range(KC):
                    xt = xts_pool.tile([128, 2 * CQ], BF16, tag="xt",
                                       name=f"xt_{t0}_{kc}")
                    nc.sync.dma_start(
                        xt[:], xT[kc * 128:(kc + 1) * 128,
                                  t0 * CQ:(t0 + 2) * CQ])
                    xts.append(xt)

                def qk_unit(w_sb, dst_sb, hf):
                    t_ = t0 + hf
                    def emit():
                        ps = sc_pool.tile([128, CQ], F32, tag="sc",
                                          name=f"qkps{t_}_{id(w_sb)}")
                        for kc in range(KC):
                            nc.tensor.matmul(
                                ps[:], w_sb[:, kc],
                                xts[kc][:, hf * CQ:(hf + 1) * CQ],
                                start=(kc == 0), stop=(kc == KC - 1))
                        if t_ < 4:
                            nc.scalar.copy(
                                dst_sb[:, t_ * CQ:(t_ + 1) * CQ], ps[:])
                        else:
                            nc.vector.tensor_copy(
                                dst_sb[:, t_ * CQ:(t_ + 1) * CQ], ps[:])
                    return emit

                def v_unit(hf):
                    t_ = t0 + hf
                    def emit():
                        v_ps = vps.tile([128, CQ], F32, tag="v",
                                        name=f"vps{t_}")
                        for j in range(CQ // 128):
                            jf = hf * CQ + j * 128
                            for kc in range(KC):
                                nc.tensor.matmul(
                                    v_ps[:, j * 128:(j + 1) * 128],
                                    xts[kc][:, jf:jf + 128],
                                    wv_sb[:, kc], start=(kc == 0),
                                    stop=(kc == KC - 1))
                        b4 = t_ * (CQ // 128)
                        nc.vector.tensor_copy(
                            v_sb[:, b4:b4 + 4, :, 0:64],
                            v_ps[:].rearrange("p (j h v) -> p j h v",
                                              j=4, h=2))
                    return emit

                return [qk_unit(wq_sb, qt_sb, 0), qk_unit(wk_sb, kt_sb, 0),
                        v_unit(0), qk_unit(wq_sb, qt_sb, 1),
                        qk_unit(wk_sb, kt_sb, 1), v_unit(1)]

        first = True
        for t0 in (0, 2):  # batch-0 projections first
            units0 = proj_pair_units(t0)  # emits the pair's xT DMAs
            if first:
                load_late_weights()
                first = False
            for u in units0:
                u()
        b1_units = None
        for b in range(B):
            if True:
                tb = b * T  # token offset of this batch
                for icq, cq in enumerate(range(NQB) if b == 0
                                         else range(NQB - 1, -1, -1)):
                    if b == 0 and icq in (0, 2):
                        b1_units = proj_pair_units(T // CQ + icq)
                    units = (b1_units[(icq % 2) * 3:(icq % 2) * 3 + 3]
                             if b == 0 else [])
                    tq0 = cq * CQ
                    nblk = (tq0 + CQ) // 128  # causal: tk blocks needed
                    pt = [pt_pool.tile([128, NKB, CQ], BF16, tag=f"pt{h}",
                                       name=f"pt{h}_{b}_{cq}")
                          for h in range(2)]
                    pv = [pv_pool.tile([128, CQ], F32, tag="pv",
                                       name=f"pv{h}_{b}_{cq}")
                          for h in range(2)]

                    for w in range(nblk // 2):  # waves of 2 tk blocks
                        kbs = (2 * w, 2 * w + 1)
                        sc = [sc_pool.tile([128, 2 * CQ], F32, tag="sc",
                                           name=f"sc{h}_{b}_{cq}_{w}")
                              for h in range(2)]
                        for i, kb in enumerate(kbs):
                            tk0 = kb * 128
                            f0 = max(tk0 - tq0, 0)
                            for h in range(2):
                                hs = slice(h * 64, (h + 1) * 64)
                                nc.tensor.matmul(
                                    sc[h][:, i * CQ + f0:(i + 1) * CQ],
                                    kt_sb[hs, tb + tk0:tb + tk0 + 128],
                                    qt_sb[hs, tb + tq0 + f0:tb + tq0 + CQ],
                                    start=True, stop=True)
                        for h in range(2):
                            nc.scalar.activation(
                                pt[h][:, 2 * w:2 * w + 2], sc[h][:],
                                AF.Exp, scale=float(D) ** -0.5)
                        for i, kb in enumerate(kbs):
                            tk0 = kb * 128
                            s = tk0 - tq0
                            f0 = max(s, 0)  # first causally-valid tq col
                            for h in range(2):
                                if 0 <= s < CQ:  # diagonal: triangle mask
                                    m_eng = (nc.gpsimd if (b == 1 and icq >= 2)
                                             else nc.vector)
                                    m_eng.tensor_mul(
                                        pt[h][:, kb, s:s + 128],
                                        pt[h][:, kb, s:s + 128], tri_sb[:])
                                nc.tensor.matmul(
                                    pv[h][0:65, f0:CQ],
                                    v_sb[:, b * NKB + kb, h],
                                    pt[h][:, kb, f0:CQ],
                                    start=(kb == 0), stop=(kb == nblk - 1))

                        if units:
                            units.pop(0)()
                    for u in units:
                        u()
                    units = []

                    # normalize: zr = 1/Z; zb = ones x zr (broadcast); mul
                    zr = zr_pool.tile([128, 2 * CQ], mybir.dt.float16, tag="zr",
                                      name=f"zr_{b}_{cq}")
                    zb_ps = ops_pool.tile([128, CQ], F32, tag="o",
                                          name=f"zb_{b}_{cq}")
                    zb_sb = zbs_pool.tile([128, CQ], F32, tag="zbs",
                                          name=f"zbs_{b}_{cq}")
                    for h in range(2):
                        zrh = zr[64:65, h * CQ:(h + 1) * CQ]
                        with nc.allow_low_precision(
                                reason="1/Z in fp16 (2.4e-4 rel) feeds the "
                                       "K=1 broadcast matmul at full PE rate"):
                            nc.vector.reciprocal(zrh, pv[h][64:65, :])
                        nc.tensor.matmul(
                            zb_ps[h * 64:(h + 1) * 64, :],
                            ones_sb[64:65, :], zrh, start=True, stop=True)
                    if b == 1 and icq >= 2:
                        nc.scalar.copy(zb_sb[:], zb_ps[:])
                    else:
                        nc.vector.tensor_copy(zb_sb[:], zb_ps[:])
                    for h in range(2):
                        nc.vector.tensor_mul(
                            ot_sb[h * 64:(h + 1) * 64, tb + tq0:tb + tq0 + CQ],
                            pv[h][0:64, :], zb_sb[h * 64:(h + 1) * 64, :])

                    # phase 3 for this chunk
                    for j in range(CQ // 128):
                        tqg = tb + tq0 + j * 128
                        ost = ost_pool.tile([128, 1024], F32, tag="ost",
                                            name=f"ost_{b}_{cq}_{j}")
                        for eh in range(2):
                            fin_pool, fin_tag = ((sc_pool, "sc")
                                                 if b == 1 and icq == 3
                                                 else (ops_pool, "o"))
                            o_ps = fin_pool.tile(
                                [128, 512], F32, tag=fin_tag,
                                name=f"o_{b}_{cq}_{j}_{eh}")
                            nc.tensor.matmul(
                                o_ps[:], ot_sb[:, tqg:tqg + 128],
                                wo_sb[:, eh * 512:(eh + 1) * 512],
                                start=True, stop=True)
                            dst = ost[:, eh * 512:(eh + 1) * 512]
                            if b == 1 and icq >= 2 and eh == 1:
                                nc.scalar.copy(dst, o_ps[:])
                            else:
                                nc.vector.tensor_copy(dst, o_ps[:])
                        nc.sync.dma_start(out[tqg:tqg + 128, :], ost[:])

    nc.compile()
    return nc


def _host_prep(x, Wq, Wk, Wv, Wo):
    bf = ml_dtypes.bfloat16
    xT = np.ascontiguousarray(
        np.asarray(x, dtype=np.float32).reshape(BT, E).T).astype(bf)

    # tri[p, f] = 1 where kept (f >= p), applied to the diagonal 128x128
    # sub-block of P^T (tk on partitions, tq on free)
    p = np.arange(128)[:, None]
    f = np.arange(128)[None, :]
    tri = (f >= p).astype(bf)

    def perm(w):
        # [E, 128] -> [128p, kc, 128d] flattened: w[kc*128+p, d] -> out[p, kc, d]
        return np.ascontiguousarray(
            w.reshape(KC, 128, 128).transpose(1, 0, 2).reshape(128, E)).astype(bf)

    Wq = np.asarray(Wq, dtype=np.float32)
    Wk = np.asarray(Wk, dtype=np.float32)
    Wv = np.asarray(Wv, dtype=np.float32)
    Wo = np.asarray(Wo, dtype=np.float32)

    in_maps = []
    for c in range(NCORE):
        sl = slice(c * 128, (c + 1) * 128)
        in_maps.append({
            "xT": xT,
            "wq": perm(Wq[:, sl]),
            "wk": perm(Wk[:, sl]),
            "wv": perm(Wv[:, sl]),
            "wo": np.ascontiguousarray(Wo[sl, :]).astype(bf),
            "tri": tri,
        })
    return in_maps


def kernel(x, Wq, Wk, Wv, Wo, bo, _trace=False, _trace_kwargs=None):
    if "nc" not in _cache:
        _cache["nc"] = _build()
    nc = _cache["nc"]

    in_maps = _host_prep(x, Wq, Wk, Wv, Wo)
    kw = {}
    if _trace:
        kw = dict(trace=True, trace_cores=[0], **(_trace_kwargs or {}))
    res = run_bass_kernel_spmd(nc, in_maps, core_ids=list(range(NCORE)), **kw)
    _cache["last_result"] = res

    total = np.zeros((BT, E), dtype=np.float32)
    for r in res.results:
        total += r["out"]
    total += np.asarray(bo, dtype=np.float32)[None, :]
    return total.reshape(B, T, E)



# revision 22
# speedup vs baseline: 1.1458x; 1.1458x over previous
"""Multi-head causal attention (B=2, T=2048, E=1024, H=16, D=64) on 8 trn2 cores.

Sharding: tensor-parallel over heads — core c owns heads {2c, 2c+1} (a 128-wide
slice of the hidden dim). Each core computes q/k/v projections for its heads
over the full sequence, causal attention, and a partial output projection
(contraction over its 128 rows of Wo). The host sums the 8 bf16 partials + bias.

v2 dataflow (vs the 164.4us v1): the P@V matmul is flipped so the exp'd
score block P^T[tk,tq] is the *stationary* operand (full 128x128 PE
utilisation, ap=64 per matmul) and V[tk,d] moves; the result O[tq,d] is
PE-transposed back to O^T for the output projection. Z = sum(exp) comes from
ap=1 ones-matmul chains instead of a 65th V column, and 1/Z is applied
per-partition during the O copy (tensor_scalar on DVE), killing v1's
broadcast matmul. Triangle masks run on GPSIMD (SBUF-only op) to keep DVE
for PSUM copies. exp activations are per tk-block over both heads
[128,2,512-f0], causally trimmed. Projections are emitted just-in-time as
filler units inside the attention wave stream so the PE never starves:
chunk c's waves interleave the projections needed by chunk c+1/c+2.

PSUM (8 banks): sc 2x[128,2,512] (4) + O|Z accumulators 2x[128,2,2,65] (2) +
mm [128,512] x2 (2, shared by proj / out-proj / transpose tiles).

Timing signal is concourse TimelineSim (no NTFF under this axon client).
"""

import numpy as np
import ml_dtypes
from collections import deque

import concourse.bass as bass
import concourse.tile as tile
from concourse import bacc, mybir
from concourse.bass_utils import run_bass_kernel_spmd
from concourse.masks import make_identity
from contextlib import ExitStack

B, T, E, H, D = 2, 2048, 1024, 16, 64
BT = B * T            # 4096 tokens total
NCORE = 8
KC = E // 128         # contraction chunks for projections = 8
CQ = 512              # tq chunk width
NQB = T // CQ         # tq chunks per batch = 4
NKB = T // 128        # tk blocks per batch = 16

F32 = mybir.dt.float32
BF16 = mybir.dt.bfloat16
AF = mybir.ActivationFunctionType

_cache = {}


def _build():
    nc = bacc.Bacc("TRN2", target_bir_lowering=False, debug=False,
                   num_devices=NCORE)

    xT = nc.dram_tensor("xT", [E, BT], BF16, kind="ExternalInput").ap()
    wq = nc.dram_tensor("wq", [128, E], BF16, kind="ExternalInput").ap()
    wk = nc.dram_tensor("wk", [128, E], BF16, kind="ExternalInput").ap()
    wv = nc.dram_tensor("wv", [128, E], BF16, kind="ExternalInput").ap()
    wo = nc.dram_tensor("wo", [128, E], BF16, kind="ExternalInput").ap()
    tri = nc.dram_tensor("tri", [128, 128], BF16, kind="ExternalInput").ap()
    out = nc.dram_tensor("out", [BT, E], BF16, kind="ExternalOutput").ap()

    with tile.TileContext(nc) as tc, ExitStack() as ctx:
        pers = ctx.enter_context(tc.tile_pool(name="pers", bufs=1))

        wq_sb = pers.tile([128, KC, 128], BF16, tag="wq")
        wk_sb = pers.tile([128, KC, 128], BF16, tag="wk")
        wv_sb = pers.tile([128, KC, 128], BF16, tag="wv")
        wo_sb = pers.tile([128, E], BF16, tag="wo")
        tri_sb = pers.tile([128, 128], BF16, tag="tri")
        eye_sb = pers.tile([128, 128], F32, tag="eye")
        qt_sb = pers.tile([128, BT], BF16, tag="qt")    # [dims(2 heads), tok]
        kt_sb = pers.tile([128, BT], BF16, tag="kt")
        # V natural + ones col per head: [tok%128, blk, h, d|1]; the ones
        # column makes the flipped P^T-stationary PV matmul emit Z = sum(exp)
        # as output column 64 for free.
        v_sb = pers.tile([128, BT // 128, 2, 65], BF16, tag="v")

        # wq/wk queued first on the sync HWDGE queue so the first projection
        # matmuls gate on as little DMA as possible.
        nc.sync.dma_start(wq_sb[:], wq.rearrange("p (kc d) -> p kc d", kc=KC))
        nc.sync.dma_start(wk_sb[:], wk.rearrange("p (kc d) -> p kc d", kc=KC))
        nc.vector.memset(v_sb[:, :, :, 64:65], 1.0)
        make_identity(nc, eye_sb[:])

        def load_late_weights():
            nc.sync.dma_start(wv_sb[:],
                              wv.rearrange("p (kc d) -> p kc d", kc=KC))
            nc.sync.dma_start(tri_sb[:], tri[:])
            nc.sync.dma_start(wo_sb[:], wo[:])

        # SBUF pools
        xts_pool = ctx.enter_context(tc.tile_pool(name="xts", bufs=16))
        pt_pool = ctx.enter_context(tc.tile_pool(name="pt", bufs=2))
        osb_pool = ctx.enter_context(tc.tile_pool(name="osb", bufs=3))
        otsb_pool = ctx.enter_context(tc.tile_pool(name="otsb", bufs=3))
        outsb_pool = ctx.enter_context(tc.tile_pool(name="outsb", bufs=3))
        zr_pool = ctx.enter_context(tc.tile_pool(name="zr", bufs=2))

        # PSUM pools: 4 + 2 + 2 = 8 banks
        sc_pool = ctx.enter_context(tc.tile_pool(name="sc", bufs=2,
                                                 space="PSUM"))
        acc_pool = ctx.enter_context(tc.tile_pool(name="acc", bufs=1,
                                                  space="PSUM"))
        mm_pool = ctx.enter_context(tc.tile_pool(name="mm", bufs=2,
                                                 space="PSUM"))



        # ---- projection units -------------------------------------------
        def proj_pair_units(t0):
            """t0: even 512-token chunk index (0..6). Issues the pair's xT
            DMAs now; returns 6 unit callbacks (q,k,v) x (hf 0,1)."""
            xts = []
            for kc in range(KC):
                xt = xts_pool.tile([128, 2 * CQ], BF16, tag="xt",
                                   name=f"xt_{t0}_{kc}")
                nc.sync.dma_start(
                    xt[:], xT[kc * 128:(kc + 1) * 128,
                              t0 * CQ:(t0 + 2) * CQ])
                xts.append(xt)

            def qk_unit(w_sb, dst_sb, hf):
                t_ = t0 + hf
                def emit():
                    ps = mm_pool.tile([128, CQ], F32, tag="mm",
                                      name=f"qkps{t_}_{id(w_sb)}")
                    for kc in range(KC):
                        nc.tensor.matmul(
                            ps[:], w_sb[:, kc],
                            xts[kc][:, hf * CQ:(hf + 1) * CQ],
                            start=(kc == 0), stop=(kc == KC - 1))
                    nc.vector.tensor_copy(
                        dst_sb[:, t_ * CQ:(t_ + 1) * CQ], ps[:])
                return emit

            def v_unit(hf):
                t_ = t0 + hf
                def emit():
                    v_ps = mm_pool.tile([128, CQ], F32, tag="mm",
                                        name=f"vps{t_}")
                    for j in range(CQ // 128):
                        jf = hf * CQ + j * 128
                        for kc in range(KC):
                            nc.tensor.matmul(
                                v_ps[:, j * 128:(j + 1) * 128],
                                xts[kc][:, jf:jf + 128],
                                wv_sb[:, kc], start=(kc == 0),
                                stop=(kc == KC - 1))
                    b4 = t_ * (CQ // 128)
                    nc.vector.tensor_copy(
                        v_sb[:, b4:b4 + 4, :, 0:64],
                        v_ps[:].rearrange("p (j h v) -> p j h v",
                                          j=4, h=2))
                return emit

            return [qk_unit(wq_sb, qt_sb, 0), qk_unit(wk_sb, kt_sb, 0),
                    v_unit(0), qk_unit(wq_sb, qt_sb, 1),
                    qk_unit(wk_sb, kt_sb, 1), v_unit(1)]

        # ---- filler machinery -------------------------------------------
        # proj_q entries are (token_chunk, callback): the unit MUST be
        # emitted before the attention chunk that consumes that token chunk
        # (a later emission would deadlock the in-order PE queue).
        tails_q = deque()
        proj_q = deque()
        dma_pending = []   # (dram_slice, sbuf_tile): out DMAs deferred one
                           # tail so the SP queue never stalls on copy sems

        def flush_out_dma():
            while dma_pending:
                dst, src = dma_pending.pop(0)
                nc.sync.dma_start(dst, src)

        def pop_fillers():
            if tails_q and (not proj_q or len(tails_q) >= 2):
                tails_q.popleft()()
            if proj_q:
                proj_q.popleft()[1]()

        def drain_tails():
            while tails_q:
                tails_q.popleft()()

        def force_proj_upto(tc_needed):
            while proj_q and proj_q[0][0] <= tc_needed:
                proj_q.popleft()[1]()

        # ---- prologue ----------------------------------------------------
        units0 = proj_pair_units(0)
        load_late_weights()
        for u in units0[:3]:      # q,k,v for tokens 0..511
            u()
        proj_q.extend((1, u) for u in units0[3:])

        # proj pair creation / unit queueing schedule, per global chunk idx:
        #   create pair(t0) => issue its xT DMAs at that chunk's start
        #   queue: which units enter proj_q at that chunk's start
        pair_create = {1: 2, 2: 4, 3: 6}
        stash = {}

        chunk_list = [(b, c) for b in range(B) for c in range(NQB)]
        for ci, (b, c) in enumerate(chunk_list):
            if ci in pair_create:
                units = proj_pair_units(pair_create[ci])
                stash[pair_create[ci]] = units[:3]
                stash[pair_create[ci] + 1] = units[3:]
            # queue units per schedule: ci->token-chunk mapping
            # c0:P1, c1:P2+P3, c2:P4, c3:P5, c4:P6, c5:P7
            queue_map = {1: [2, 3], 2: [4], 3: [5], 4: [6], 5: [7]}
            for t in queue_map.get(ci, []):
                proj_q.extend((t, u) for u in stash.pop(t))
            # anything this chunk's scores/PV depends on must be emitted now
            force_proj_upto(b * NQB + c)

            tb = b * T
            tq0 = c * CQ
            nblk = 4 * (c + 1)
            pt = pt_pool.tile([128, NKB, 2, CQ], BF16, tag="pt",
                              name=f"pt_{b}_{c}")
            zr_sb = zr_pool.tile([128, 2, 2, 2], F32, tag="zr",
                                 name=f"zr_{b}_{c}")  # [pair, gsub, h]
            # per-chunk O|Z accumulators [tq, gsub, h, d|Z]: pool rotation
            # (bufs=1) orders the next chunk's first PV write after this
            # chunk's tail reads
            o_ps = [acc_pool.tile([128, 2, 2, 65], F32, tag=f"o{i}",
                                  name=f"o_ps{i}_{b}_{c}")
                    for i in range(2)]

            def make_tail(g, b=b, c=c, tb=tb, tq0=tq0, zr_sb=zr_sb,
                          o_ps=o_ps):
                def emit():
                    op = o_ps[g // 2]
                    gs = g % 2
                    if gs == 0:
                        # 1/Z for this tq group pair (both groups/heads: the
                        # pair's chains have both stopped by emission time)
                        nc.vector.reciprocal(
                            zr_sb[:, g // 2], op[:, :, :, 64])
                    # normalized O copy (per head, per-partition 1/Z scale)
                    o_sb = osb_pool.tile([128, 128], F32, tag="osb",
                                         name=f"osb_{b}_{c}_{g}")
                    for h in range(2):
                        nc.vector.tensor_scalar_mul(
                            o_sb[:, h * 64:(h + 1) * 64],
                            op[:, gs, h, 0:64],
                            zr_sb[:, g // 2, gs, h:h + 1])
                    # transpose O[tq,d] -> O^T[d,tq] (f32, 2 cyc/row)
                    tp = mm_pool.tile([128, 512], F32, tag="mm",
                                      name=f"tp_{b}_{c}_{g}")
                    nc.tensor.transpose(tp[:, 0:128], o_sb[:], eye_sb[:])
                    ot_sb = otsb_pool.tile([128, 128], BF16, tag="otsb",
                                           name=f"otsb_{b}_{c}_{g}")
                    nc.vector.tensor_copy(ot_sb[:], tp[:, 0:128])
                    # output projection + final copy + DMA
                    out_sb = outsb_pool.tile([128, E], BF16, tag="outsb",
                                             name=f"outsb_{b}_{c}_{g}")
                    for eh in range(2):
                        ops = mm_pool.tile([128, 512], F32, tag="mm",
                                           name=f"ops_{b}_{c}_{g}_{eh}")
                        nc.tensor.matmul(
                            ops[:], ot_sb[:],
                            wo_sb[:, eh * 512:(eh + 1) * 512],
                            start=True, stop=True)
                        if b == 0 and eh == 1:
                            nc.scalar.copy(
                                out_sb[:, eh * 512:(eh + 1) * 512], ops[:])
                        else:
                            nc.vector.tensor_copy(
                                out_sb[:, eh * 512:(eh + 1) * 512], ops[:])
                    tqg = tb + tq0 + g * 128
                    flush_out_dma()
                    dma_pending.append((out[tqg:tqg + 128, :], out_sb[:]))
                return emit

            # PSUM has_written bits: a start=True matmul clears them for the
            # WHOLE bank, so only the first PV matmul per o_ps bank per chunk
            # may use start=True. Later chains' first matmuls (kb==0,
            # start=False) overwrite-where-bit-clear, then accumulate.
            bank_started = [False, False]

            def pv_block(kb, b=b, c=c, pt=pt, o_ps=o_ps,
                         bank_started=bank_started):
                j0 = max(0, kb - 4 * c)
                for g in range(j0, NQB):
                    for h in range(2):
                        st = not bank_started[g // 2]
                        bank_started[g // 2] = True
                        nc.tensor.matmul(
                            o_ps[g // 2][:, g % 2, h, :],
                            pt[:, kb, h, g * 128:(g + 1) * 128],
                            v_sb[:, b * NKB + kb, h],
                            start=st, stop=(kb == 4 * c + g),
                            skip_group_check=True)
                j = kb - 4 * c
                if j in (1, 3):  # group pair's chains complete
                    tails_q.append(make_tail(j - 1))
                    tails_q.append(make_tail(j))

            for kb in range(nblk):
                f0 = max(0, 128 * (kb - 4 * c))
                sc = sc_pool.tile([128, 2, CQ], F32, tag="sc",
                                  name=f"sc_{b}_{c}_{kb}")
                tk0 = kb * 128
                for h in range(2):
                    hs = slice(h * 64, (h + 1) * 64)
                    nc.tensor.matmul(
                        sc[:, h, f0:CQ],
                        kt_sb[hs, tb + tk0:tb + tk0 + 128],
                        qt_sb[hs, tb + tq0 + f0:tb + tq0 + CQ],
                        start=True, stop=True)
                nc.scalar.activation(
                    pt[:, kb, :, f0:CQ], sc[:, :, f0:CQ],
                    AF.Exp, scale=float(D) ** -0.5)
                if kb - 4 * c >= 0:  # diagonal block: triangle mask
                    for h in range(2):
                        nc.gpsimd.tensor_mul(
                            pt[:, kb, h, f0:f0 + 128],
                            pt[:, kb, h, f0:f0 + 128], tri_sb[:])
                if kb >= 1:
                    if kb == 1:
                        drain_tails()  # prev chunk's tails before 1st pv
                    pv_block(kb - 1)
                pop_fillers()
            pv_block(nblk - 1)

        drain_tails()
        while proj_q:
            proj_q.popleft()[1]()
        flush_out_dma()

    nc.compile()
    return nc


def _host_prep(x, Wq, Wk, Wv, Wo):
    bf = ml_dtypes.bfloat16
    xT = np.ascontiguousarray(
        np.asarray(x, dtype=np.float32).reshape(BT, E).T).astype(bf)

    # tri[p, f] = 1 where kept (f >= p), applied to the diagonal 128x128
    # sub-block of P^T (tk on partitions, tq on free)
    p = np.arange(128)[:, None]
    f = np.arange(128)[None, :]
    tri = (f >= p).astype(bf)

    def perm(w):
        # [E, 128] -> [128p, kc, 128d] flattened: w[kc*128+p, d] -> out[p, kc, d]
        return np.ascontiguousarray(
            w.reshape(KC, 128, 128).transpose(1, 0, 2).reshape(128, E)).astype(bf)

    Wq = np.asarray(Wq, dtype=np.float32)
    Wk = np.asarray(Wk, dtype=np.float32)
    Wv = np.asarray(Wv, dtype=np.float32)
    Wo = np.asarray(Wo, dtype=np.float32)

    in_maps = []
    for c in range(NCORE):
        sl = slice(c * 128, (c + 1) * 128)
        in_maps.append({
            "xT": xT,
            "wq": perm(Wq[:, sl]),
            "wk": perm(Wk[:, sl]),
            "wv": perm(Wv[:, sl]),
            "wo": np.ascontiguousarray(Wo[sl, :]).astype(bf),
            "tri": tri,
        })
    return in_maps


def kernel(x, Wq, Wk, Wv, Wo, bo, _trace=False, _trace_kwargs=None):
    if "nc" not in _cache:
        _cache["nc"] = _build()
    nc = _cache["nc"]

    in_maps = _host_prep(x, Wq, Wk, Wv, Wo)
    kw = {}
    if _trace:
        kw = dict(trace=True, trace_cores=[0], **(_trace_kwargs or {}))
    res = run_bass_kernel_spmd(nc, in_maps, core_ids=list(range(NCORE)), **kw)
    _cache["last_result"] = res

    total = np.zeros((BT, E), dtype=np.float32)
    for r in res.results:
        total += np.asarray(r["out"], dtype=np.float32)
    total += np.asarray(bo, dtype=np.float32)[None, :]
    return total.reshape(B, T, E)
